# revision 31
# baseline (speedup 1.0000x reference)
"""LoRA MultiheadAttention on 8 NeuronCores (Bass/Tile), v5.

Sharding: 32 (batch, head) attention slices -> 4 heads x 1 batch per core.
Cores 0-3 take batch 0, cores 4-7 batch 1; core c handles heads
(c%4)*4 .. (c%4)*4+3, i.e. a contiguous 256-wide slice of the head dims.

The PE is drain-bound on TRN2 (every matmul costs N fp32-PSUM-drain columns
at 1 col/cycle regardless of K/M), so the kernel keeps the PE instruction
stream dense end-to-end (all matmuls bf16; fp8 was tried and rejected:
e4m3's ~4% per-element noise does not average away in random GEMMs):

  prologue: A^T LoRA activations, all of Q^T/K^T, V row-tiles 0-2.
  attention: 8 single-head units x 16 tj iterations x 2 512-wide score
             chunks. 2 chunks/iter against 3 pS slots leaves a full exp of
             cross-iteration slack, so score matmuls rarely wait. The
             remaining 13 V row-tiles (paced ahead of their PV consumers)
             and the half-0 out-projection are drained into the PE stream
             as filler so exp waits never idle the PE; filler accumulates
             in a single spare PSUM bank.
  exp split: ACT (real exp) and DVE (one-op Schraudolph bf16:
             i16 = rint(s*128/ln2 + B) bitcast bf16, mean-zero calibrated,
             ~40% of chunks; softmax renormalizes, output err ~0.5%).
  norm:      po evacuated to bf16 SBUF immediately (frees PSUM banks);
             denominator row round-trips through DRAM reshaped to [128, 8]
             so the reciprocal uses 128 DVE lanes (0.13us vs 6.5us for a
             [64,1024] broadcast reciprocal); stride-0 DMA broadcast; one
             2x-mode bf16 multiply into oT_sb.
  epilogue:  half-1 out-projection, PSUM->SBUF copies alternating ACT/DVE.

b_v is folded into the V matmul ones-row bias; out_b added on host.
"""

import sys

sys.path.insert(0, "/opt/trn_rl_repo")

import math
from contextlib import ExitStack

import ml_dtypes
import numpy as np

import concourse.bass as bass
import concourse.tile as tile
from concourse import mybir
from concourse.alu_op_type import AluOpType
from concourse.bass_utils import run_bass_kernel_spmd

BF16 = ml_dtypes.bfloat16
F32 = mybir.dt.float32
BF = mybir.dt.bfloat16
I16 = mybir.dt.int16

T = 2048
D = 1024
H = 16
HD = 64
R = 16
BSZ = 2
SCALE = 16.0
NCORES = 8
HPC = 4  # heads per core
CD = HPC * HD  # 256 head dims per core
VW = HD + 1  # V block width per head (ones column appended)
KPAD = 1152  # 1024 X rows + 1 ones row, padded to 9 k-tiles of 128
NKT = KPAD // 128
P = 128
NTT = T // P  # 16 row tiles
HF = T // 2  # 1024: ti processed in two halves

# Schraudolph-bf16 exp: i16 = rint(x * 128/ln2 + (127*128 - C)), bitcast bf16
EXP_A = 128.0 / math.log(2.0)
EXP_B = 127.0 * 128.0 - 7.3
# tj tiles whose c==1 exp chunk goes to DVE-Schraudolph (rest go to ACT)
DVE_TJ = frozenset(range(16)) - {5, 10, 15}


def build_nc():
    nc = bass.Bass()
    xa = nc.dram_tensor("xa", [NKT, P, T], BF, kind="ExternalInput")
    wqk = nc.dram_tensor("wqk", [NKT, P, 2 * CD], BF, kind="ExternalInput")
    wv = nc.dram_tensor("wv", [NKT, P, HPC * VW], BF, kind="ExternalInput")
    ab = nc.dram_tensor("ab", [NKT, P, 3 * R], BF, kind="ExternalInput")
    kbm = nc.dram_tensor("kbm", [R, CD], BF, kind="ExternalInput")
    vbm = nc.dram_tensor("vbm", [R, HPC * VW], BF, kind="ExternalInput")
    wo = nc.dram_tensor("wo", [2, P, D], BF, kind="ExternalInput")
    out = nc.dram_tensor("out", [NTT, P, D], BF, kind="ExternalOutput")

    with tile.TileContext(nc) as tc, ExitStack() as ctx:
        singles = ctx.enter_context(tc.tile_pool(name="singles", bufs=1))

        xa_t = [singles.tile([P, T], BF, name=f"xa{i}", tag=f"xa{i}") for i in range(NKT)]
        wqk_t = [singles.tile([P, 2 * CD], BF, name=f"wqk{i}", tag=f"wqk{i}") for i in range(NKT)]
        wv_t = [singles.tile([P, HPC * VW], BF, name=f"wv{i}", tag=f"wv{i}") for i in range(NKT)]
        ab_t = [singles.tile([P, 3 * R], BF, name=f"ab{i}", tag=f"ab{i}") for i in range(NKT)]
        kb_t = singles.tile([P, CD], BF, tag="kb")
        vb_t = singles.tile([P, HPC * VW], BF, tag="vb")
        nc.vector.memset(kb_t, 0.0)
        nc.vector.memset(vb_t, 0.0)
        wo_t = [singles.tile([P, D], BF, name=f"wo{i}", tag=f"wo{i}") for i in range(2)]
        # load order matches consumption: A needs ab+xa, then B needs wqk
        for c4 in range(4):
            nc.sync.dma_start(
                out=xa_t[0][:, c4 * 512 : (c4 + 1) * 512],
                in_=xa[0, :, c4 * 512 : (c4 + 1) * 512],
            )
            if c4 == 0:
                nc.sync.dma_start(out=ab_t[0], in_=ab[0, :, :])
        nc.sync.dma_start(out=wqk_t[0], in_=wqk[0, :, :])
        for i in range(1, NKT):
            nc.sync.dma_start(out=ab_t[i], in_=ab[i, :, :])
            nc.sync.dma_start(out=xa_t[i], in_=xa[i, :, :])
            nc.sync.dma_start(out=wqk_t[i], in_=wqk[i, :, :])
        for i in range(NKT):
            nc.sync.dma_start(out=wv_t[i], in_=wv[i, :, :])
        nc.sync.dma_start(out=kb_t[0:R, :], in_=kbm[:, :])
        nc.sync.dma_start(out=vb_t[0:R, :], in_=vbm[:, :])
        for i in range(2):
            nc.sync.dma_start(out=wo_t[i], in_=wo[i, :, :])

        # Q^T tiles (heads 0-1 / 2-3); K^T stored per head zero-padded to
        # 128 contraction rows so every attention matmul runs in the PE's
        # (128,128) tiling mode -- mode switches drain the whole array.
        qk_sb = [singles.tile([P, T], BF, name=f"qk{i}", tag=f"qk{i}") for i in range(2)]
        kp_sb = [singles.tile([P, T], BF, name=f"kp{i}", tag=f"kp{i}") for i in range(HPC)]
        ak_sb = singles.tile([P, T], BF, tag="ak")
        av_sb = singles.tile([P, T], BF, tag="av")
        for t8 in kp_sb:
            nc.vector.memset(t8, 0.0)
        nc.vector.memset(ak_sb, 0.0)
        nc.vector.memset(av_sb, 0.0)
        v_sb = [singles.tile([P, HPC * VW], BF, name=f"v{i}", tag=f"v{i}") for i in range(NTT)]
        oT_sb = [singles.tile([P, T], BF, name=f"oT{i}", tag=f"oT{i}") for i in range(2)]
        pOut = ctx.enter_context(tc.tile_pool(name="pOut", bufs=3))
        ob_sb = {}

        def emit_b_chunk(pool, m, ch, copy_engine):
            cs = slice(ch * 512, (ch + 1) * 512)
            pq = pool.tile([P, 512], F32, tag="aux", name=f"pq_{m}_{ch}")
            steps = []
            for kt in range(NKT):
                steps.append(
                    lambda kt=kt, pq=pq: nc.tensor.matmul(
                        pq,
                        lhsT=wqk_t[kt][:, m * P : (m + 1) * P],
                        rhs=xa_t[kt][:, cs],
                        start=(kt == 0),
                        stop=(kt == NKT - 1 and m < 2),
                    )
                )
            if m >= 2:
                steps.append(
                    lambda pq=pq: nc.tensor.matmul(
                        pq,
                        lhsT=kb_t[:, (m - 2) * P : (m - 1) * P],
                        rhs=ak_sb[:, cs],
                        start=False,
                        stop=True,
                    )
                )
                h0 = 2 * (m - 2)
                steps.append(
                    lambda pq=pq, h0=h0: copy_engine(
                        kp_sb[h0][0:HD, cs], pq[0:HD, :]
                    )
                )
                steps.append(
                    lambda pq=pq, h0=h0: copy_engine(
                        kp_sb[h0 + 1][HD:P, cs], pq[HD:P, :]
                    )
                )
            else:
                steps.append(lambda pq=pq: copy_engine(qk_sb[m][:, cs], pq))
            return steps

        def emit_c_chunk(pool, mt, copy_engine):
            ms = slice(mt * P, (mt + 1) * P)
            pv = pool.tile([P, HPC * VW], F32, tag="aux", name=f"pv_{mt}")
            steps = []
            for kt in range(NKT):
                steps.append(
                    lambda kt=kt, pv=pv: nc.tensor.matmul(
                        pv,
                        lhsT=xa_t[kt][:, ms],
                        rhs=wv_t[kt],
                        start=(kt == 0),
                        stop=False,
                    )
                )
            steps.append(
                lambda pv=pv: nc.tensor.matmul(
                    pv, lhsT=av_sb[:, ms], rhs=vb_t, start=False, stop=True
                )
            )
            steps.append(lambda pv=pv: copy_engine(v_sb[mt], pv))
            return steps

        def emit_outproj_chunk(pool, mt, ch, copy_engine):
            ms = slice(mt * P, (mt + 1) * P)
            cs = slice(ch * 512, (ch + 1) * 512)
            po2 = pool.tile([P, 512], F32, tag="aux", name=f"po2_{mt}_{ch}")
            steps = []
            if ch == 0:
                def mkob(mt=mt):
                    ob_sb[mt] = pOut.tile([P, D], BF, tag="ob", name=f"ob_{mt}")
                steps.append(mkob)
            for kt2 in range(2):
                steps.append(
                    lambda kt2=kt2, po2=po2: nc.tensor.matmul(
                        po2,
                        lhsT=oT_sb[kt2][:, ms],
                        rhs=wo_t[kt2][:, cs],
                        start=(kt2 == 0),
                        stop=(kt2 == 1),
                    )
                )

            def fin(po2=po2, mt=mt, ch=ch):
                copy_engine(ob_sb[mt][:, cs], po2)
                if ch == 1:
                    nc.sync.dma_start(out=out[mt, :, :], in_=ob_sb[mt])

            steps.append(fin)
            return steps

        def pro_copy(i=[0]):
            i[0] += 1
            return nc.vector.tensor_copy if i[0] % 2 else nc.scalar.copy

        # Phase A + K^T(m2) prologue, kt-major: each k-tile of every
        # accumulation chain is consumed as soon as its DMA lands, so the
        # PE tracks the input load instead of waiting for it.
        with tc.tile_pool(name="pA", bufs=4, space="PSUM") as pA, \
             tc.tile_pool(name="pPro", bufs=4, space="PSUM") as pPro:
            pa4 = [pA.tile([3 * R, 512], F32, tag="pa", name=f"pa{ch}") for ch in range(4)]
            bch = [emit_b_chunk(pPro, 2, ch, pro_copy()) for ch in range(4)]
            for kt in range(NKT):
                if kt < 8:  # ab rows >= 1024 are zero; skip 9th tile
                    for ch in range(4):
                        nc.tensor.matmul(
                            pa4[ch],
                            lhsT=ab_t[kt],
                            rhs=xa_t[kt][:, ch * 512 : (ch + 1) * 512],
                            start=(kt == 0),
                            stop=(kt == 7),
                        )
                for c in bch:
                    c[kt]()
            for ch in range(4):
                cs = slice(ch * 512, (ch + 1) * 512)
                nc.vector.tensor_copy(ak_sb[0:R, cs], pa4[ch][0:R, :])
                nc.vector.tensor_copy(av_sb[0:R, cs], pa4[ch][2 * R : 3 * R, :])
            for c in bch:
                for step in c[NKT:]:
                    step()
        with tc.tile_pool(name="pPro2", bufs=3, space="PSUM") as pPro2:
            for ch in range(2):
                for step in emit_b_chunk(pPro2, 0, ch, pro_copy()):
                    step()
            for mt in range(3):
                for step in emit_c_chunk(pPro2, mt, pro_copy()):
                    step()

        # Attention: 8 single-head units with interleaved filler. Only the
        # PSUM pools live in this block; the norm-path SBUF/DRAM pools are
        # outer-scope so the block-close barrier doesn't serialize on the
        # final norm's DMA round-trips.
        pP = ctx.enter_context(tc.tile_pool(name="pP", bufs=8))
        pEv = ctx.enter_context(tc.tile_pool(name="pEv", bufs=3))
        pN = ctx.enter_context(tc.tile_pool(name="pN", bufs=3))
        pD = ctx.enter_context(tc.tile_pool(name="pD", bufs=3, space="DRAM"))
        with (
            tc.tile_pool(name="pS", bufs=4, space="PSUM") as pS,
            tc.tile_pool(name="pO", bufs=3, space="PSUM") as pO,
            tc.tile_pool(name="pX", bufs=1, space="PSUM") as pX,
        ):
            filler = []

            def alt_copy(i=[0]):
                i[0] += 1
                return nc.vector.tensor_copy if i[0] % 2 else nc.scalar.copy

            # queue order respects consumer deadlines: C[mt] before unit
            # (0,0)'s PV(mt); m3+m1(half0) before unit (0,2); m0(half1)
            # before unit (1,0); m1(half1) before unit (1,2)
            for mt in range(3, NTT):
                filler.extend(emit_c_chunk(pX, mt, alt_copy()))
            for m, ch in [(3, 0), (3, 1), (3, 2), (3, 3), (1, 0), (1, 1),
                          (0, 2), (0, 3), (1, 2), (1, 3)]:
                filler.extend(emit_b_chunk(pX, m, ch, alt_copy()))
            fill_pos = [0]

            def drain_filler(n):
                i = fill_pos[0]
                for _ in range(n):
                    if i >= len(filler):
                        break
                    filler[i]()
                    i += 1
                fill_pos[0] = i

            def emit_unit(half, h, fill_rate):
                hp = h // 2
                po = [
                    pO.tile([VW, 512], F32, tag="po", name=f"po_{half}_{h}_{c}")
                    for c in range(2)
                ]
                pts = {}

                def emit_pv(t):
                    pt = pts.pop(t)
                    for c in range(2):
                        nc.tensor.matmul(
                            po[c],
                            lhsT=v_sb[t][:, h * VW : (h + 1) * VW],
                            rhs=pt[:, c * 512 : (c + 1) * 512],
                            start=(t == 0),
                            stop=(t == NTT - 1),
                        )

                for tj in range(NTT):
                    pt = pP.tile([P, HF], BF, tag="pt", name=f"pt_{half}_{h}_{tj}")
                    for c in range(2):
                        s_ps = pS.tile([P, 512], F32, tag="s", name=f"ps_{half}_{h}_{tj}_{c}")
                        nc.tensor.matmul(
                            s_ps,
                            lhsT=kp_sb[h][:, tj * P : (tj + 1) * P],
                            rhs=qk_sb[hp][:, half * HF + c * 512 : half * HF + (c + 1) * 512],
                            start=True,
                            stop=True,
                        )
                        ptc = pt[:, c * 512 : (c + 1) * 512]
                        if c == 1 and tj in DVE_TJ:
                            nc.vector.tensor_scalar(
                                ptc.bitcast(I16), s_ps, EXP_A, EXP_B,
                                AluOpType.mult, AluOpType.add,
                            )
                        else:
                            nc.scalar.activation(
                                ptc, s_ps, mybir.ActivationFunctionType.Exp
                            )
                    pts[tj] = pt
                    drain_filler(fill_rate)
                    if tj > 1:
                        emit_pv(tj - 2)
                emit_pv(NTT - 2)
                emit_pv(NTT - 1)
                return po

            def emit_norm_rest(half, h, ev):
                hs = slice(half * HF, (half + 1) * HF)
                # SBUF->SBUF DMAs: spread the denominator row across 128
                # partitions, reciprocal on all lanes, reshape back, then
                # stride-0 broadcast -- one DMA hop fewer than via DRAM
                den128 = pN.tile([P, HF // P], BF, tag="d128", name=f"d128_{half}_{h}")
                nc.sync.dma_start(out=den128, in_=ev[HD:VW, :])
                rec = pN.tile([P, HF // P], BF, tag="rec", name=f"rec_{half}_{h}")
                with nc.allow_low_precision(
                    reason="softmax denom ~2048; bf16 recip adds ~0.4% row scale noise"
                ):
                    nc.vector.reciprocal(rec, den128)
                rw = pD.tile([1, HF], BF, tag="rw", name=f"rw_{half}_{h}")
                nc.sync.dma_start(
                    out=bass.AP(tensor=rw.tensor, offset=rw.offset,
                                ap=[[HF // P, P], [1, HF // P]]),
                    in_=rec,
                )
                rb = pN.tile([HD, HF], BF, tag="rb", name=f"rb_{half}_{h}")
                nc.sync.dma_start(
                    out=rb,
                    in_=bass.AP(tensor=rw.tensor, offset=rw.offset,
                                ap=[[0, HD], [1, HF]]),
                )
                nc.vector.tensor_mul(
                    oT_sb[h // 2][(h % 2) * HD : (h % 2) * HD + HD, hs],
                    ev[0:HD, :],
                    rb,
                )

            def emit_evac(half, h, po):
                ev = pEv.tile([VW, HF], BF, tag="ev", name=f"ev_{half}_{h}")
                for c in range(2):
                    nc.vector.tensor_copy(ev[:, c * 512 : (c + 1) * 512], po[c])
                return ev

            def emit_norm(half, h, po):
                emit_norm_rest(half, h, emit_evac(half, h, po))

            prev = None
            for half in range(2):
                for h in range(HPC):
                    rate = {(0, 0): 9, (0, 1): 6, (0, 2): 3}.get((half, h), 2)
                    po = emit_unit(half, h, rate)
                    if prev is not None:
                        emit_norm(*prev)
                        if (half, h) == (1, 0):
                            for mt in range(NTT // 2):
                                for ch in range(2):
                                    filler.extend(
                                        emit_outproj_chunk(pX, mt, ch, alt_copy())
                                    )
                    prev = (half, h, po)
            last_ev = (prev[0], prev[1], emit_evac(*prev))
            drain_filler(len(filler))

        # Epilogue: finish the last unit's norm outside the attention
        # pools (so the pool-close barrier doesn't serialize on its DMA
        # chain), then the half-1 out-projection in two 8-chunk waves --
        # each wave's kt2=0 matmuls depend only on the early half-1 norms
        # and bridge the remaining norm latency.
        with tc.tile_pool(name="pE", bufs=8, space="PSUM") as pE:
            eng = [nc.vector.tensor_copy, nc.scalar.copy]
            for wave in range(2):
                mts = range(NTT // 2 + wave * 4, NTT // 2 + wave * 4 + 4)
                chunks = [(mt, ch) for mt in mts for ch in range(2)]
                po2s = {
                    (mt, ch): pE.tile([P, 512], F32, tag="aux", name=f"po2e_{mt}_{ch}")
                    for (mt, ch) in chunks
                }
                for (mt, ch) in chunks:
                    nc.tensor.matmul(
                        po2s[(mt, ch)],
                        lhsT=oT_sb[0][:, mt * P : (mt + 1) * P],
                        rhs=wo_t[0][:, ch * 512 : (ch + 1) * 512],
                        start=True,
                        stop=False,
                    )
                if wave == 0:
                    # last norm's DMA chain emitted AFTER the kt2=0 pass so
                    # semaphore coarsening can't attach it to those matmuls;
                    # the kt2=1 pass below then carries the real dependency
                    emit_norm_rest(*last_ev)
                for (mt, ch) in chunks:
                    nc.tensor.matmul(
                        po2s[(mt, ch)],
                        lhsT=oT_sb[1][:, mt * P : (mt + 1) * P],
                        rhs=wo_t[1][:, ch * 512 : (ch + 1) * 512],
                        start=False,
                        stop=True,
                    )
                for j, (mt, ch) in enumerate(chunks):
                    if ch == 0:
                        ob_sb[mt] = pOut.tile([P, D], BF, tag="ob", name=f"ob_{mt}")
                    eng[j % 2](ob_sb[mt][:, ch * 512 : (ch + 1) * 512], po2s[(mt, ch)])
                    if ch == 1:
                        nc.sync.dma_start(out=out[mt, :, :], in_=ob_sb[mt])

    import bass_rust as _bass_rust

    _bass_rust.move_matmul_waits_to_ldweights(nc.m)
    _bass_rust.generate_event_semaphores(nc)
    return nc


def prepare_in_maps(inputs):
    q = np.asarray(inputs["query"], np.float32)
    ipw = np.asarray(inputs["in_proj_weight"], np.float32)
    ipb = np.asarray(inputs["in_proj_bias"], np.float32)
    out_w = np.asarray(inputs["out_w"], np.float32)
    k_a = np.asarray(inputs["k_a"], np.float32)
    k_b = np.asarray(inputs["k_b"], np.float32)
    v_a = np.asarray(inputs["v_a"], np.float32)
    v_b = np.asarray(inputs["v_b"], np.float32)
    qscale = 1.0 / math.sqrt(HD)
    sl = SCALE / R

    in_maps = []
    for c in range(NCORES):
        bb = c // 4
        s = (c % 4) * CD
        e = s + CD
        X = q[:, bb, :]

        xa = np.zeros((KPAD, T), np.float32)
        xa[:D] = X.T
        xa[D] = 1.0

        wqk = np.zeros((KPAD, 2 * CD), np.float32)
        wqk[:D, :CD] = ipw[s:e].T * qscale
        wqk[D, :CD] = ipb[s:e] * qscale
        wqk[:D, CD:] = ipw[D + s : D + e].T
        wqk[D, CD:] = ipb[D + s : D + e]

        wv = np.zeros((KPAD, HPC * VW), np.float32)
        for j in range(HPC):
            wv[:D, j * VW : j * VW + HD] = ipw[2 * D + s + j * HD : 2 * D + s + (j + 1) * HD].T
            wv[D, j * VW : j * VW + HD] = ipb[2 * D + s + j * HD : 2 * D + s + (j + 1) * HD]
            wv[D, j * VW + HD] = 1.0

        ab = np.zeros((KPAD, 3 * R), np.float32)
        ab[:D, :R] = k_a.T
        ab[:D, 2 * R :] = v_a.T

        kbm = k_b[:, s:e] * sl

        vbm = np.zeros((R, HPC * VW), np.float32)
        for j in range(HPC):
            vbm[:, j * VW : j * VW + HD] = v_b[:, s + j * HD : s + (j + 1) * HD] * sl

        wo = out_w[:, s:e].T

        in_maps.append(
            {
                "xa": xa.astype(BF16).reshape(NKT, P, T),
                "wqk": wqk.astype(BF16).reshape(NKT, P, 2 * CD),
                "wv": wv.astype(BF16).reshape(NKT, P, HPC * VW),
                "ab": ab.astype(BF16).reshape(NKT, P, 3 * R),
                "kbm": kbm.astype(BF16),
                "vbm": vbm.astype(BF16),
                "wo": wo.astype(BF16).reshape(2, P, D),
            }
        )
    return in_maps


def assemble_output(inputs, results):
    out_b = np.asarray(inputs["out_b"], np.float32)
    out = np.zeros((T, BSZ, D), np.float32)
    for c in range(NCORES):
        out[:, c // 4, :] += results[c]["out"].astype(np.float32).reshape(T, D)
    out += out_b[None, None, :]
    return out


def kernel(**inputs):
    nc = build_nc()
    in_maps = prepare_in_maps(inputs)
    res = run_bass_kernel_spmd(nc, in_maps, core_ids=list(range(NCORES)))
    return assemble_output(inputs, res.results)
